# revision 12
# baseline (speedup 1.0000x reference)
"""GAT (2-layer, PyG-style) on 8 Trainium2 NeuronCores via Bass/Tile.

v2 strategy (chunked 1-node-row gathers, no on-chip unpacking):
- Nodes sharded by dst across 8 cores; per core dsts are degree-sorted into
  98 tiles of 128 (partition = dst). Node features live in tables whose row
  order is this global degree-sorted position ("gpos"); x is permuted to
  gpos order host-side and pre-transposed, so the node phase is a straight
  matmul and the self-loop rows are contiguous.
- dma_gather rows must be 256B multiples and int16-indexed (<=32768 rows),
  so the edge phase runs as 4 passes over src-gpos chunks of 25088 rows,
  each gathering 1-node 256B rows directly (no 4-pack + one-hot select).
  Per-chunk degree-sorted dst tiling keeps slot padding at ~2.6%.
- Each chunk pass accumulates partial [den|num] per dst in its own dst
  order; a combine phase gathers the 4 partials back to common order, adds
  the self-loop term, and finalizes (softmax divide, elu, layer-2 linear).
- Layer 2 re-uses the exact same index streams (same edges, same slots)
  against a [NPAD, 256B] table of (al2_src, y0, y1, al2_dst) rows.
- Node phase is sharded (each core computes its 12544 rows) and shards are
  exchanged with AllGather.
"""

import numpy as np
import ml_dtypes

BF16 = ml_dtypes.bfloat16

N = 100_000
E = 3_200_000
IN = 128
H1, C1 = 8, 8
HID = H1 * C1          # 64
OUT = 2
NEG = 0.2
NCORES = 8
ND = N // NCORES       # real dsts per core: 12500
NT = 98                # tiles per core (98*128 = 12544)
PT = NT * 128          # padded dst slots per core
NPAD = PT * NCORES     # 100352
NCH = 4                # src chunks
CH = NPAD // NCH       # 25088 rows per chunk (int16-addressable)
EW = 128               # table row elems (bf16) = 256B
import os as _os
NIMAX = int(_os.environ.get("GAT_NIMAX", "2048"))  # idx per dma_gather call
BUFC = 96              # gather-buffer columns (tile-aligned packing limit)
RUNC = 64              # max columns per compute run
FG = 25                # finalize group cols
IXW = 512              # idx tile width (int16 cols; cmb calls use 392)
USE_PRELU = True


def _wrap_idx(flat):
    """int16 index array -> [128, n/16] wrapped-in-16-partitions, replicated x8."""
    n = flat.shape[0]
    assert n % 16 == 0
    w = flat.reshape(n // 16, 16).T            # [16, n/16]
    return np.tile(w, (8, 1)).astype(np.int16)  # [128, n/16]


def _plan(src, dst):
    """Host-side planning. Returns per-core streams + the common schedule."""
    core = dst // ND
    dloc = dst % ND

    # common (total-degree) order per core -> gpos
    deg_tot = np.zeros((NCORES, ND), dtype=np.int64)
    for c in range(NCORES):
        deg_tot[c] = np.bincount(dloc[core == c], minlength=ND)
    perm = np.full((NCORES, PT), -1, dtype=np.int64)
    gposl = np.zeros((NCORES, ND), dtype=np.int64)  # dloc -> common position
    for c in range(NCORES):
        order = np.argsort(-deg_tot[c], kind="stable")
        perm[c, :ND] = order
        gposl[c, order] = np.arange(ND)
    gpos = np.zeros(N, dtype=np.int64)
    for c in range(NCORES):
        gpos[c * ND + perm[c, :ND]] = c * PT + np.arange(ND)

    src_gpos = gpos[src]
    chunk = src_gpos // CH

    # per-(core, chunk) degree-sorted slot assignment
    permk = np.full((NCORES, NCH, PT), -1, dtype=np.int64)
    poslk = np.zeros((NCORES, NCH, ND), dtype=np.int64)
    degk_all = np.zeros((NCORES, NCH, ND), dtype=np.int64)
    for c in range(NCORES):
        for k in range(NCH):
            m = (core == c) & (chunk == k)
            degk = np.bincount(dloc[m], minlength=ND)
            degk_all[c, k] = degk
            ok = np.argsort(-degk, kind="stable")
            permk[c, k, :ND] = ok
            poslk[c, k, ok] = np.arange(ND)

    # common K schedule: K[k, t] = max in-chunk degree over cores in tile t
    K = np.zeros((NCH, NT), dtype=np.int64)
    for k in range(NCH):
        for t in range(NT):
            mx = 1
            for c in range(NCORES):
                sl = permk[c, k, t * 128 : (t + 1) * 128]
                real = sl[sl >= 0]
                if real.size:
                    mx = max(mx, int(degk_all[c, k, real].max()))
            K[k, t] = mx
    col0 = np.zeros((NCH, NT), dtype=np.int64)
    for k in range(NCH):
        col0[k] = np.cumsum(K[k]) - K[k]
    ncols_k = K.sum(axis=1)

    # buffer packing (tile-aligned, <= BUFC cols) and gather-call layout
    buffers = []   # (k, t_lo, t_hi, cols)
    for k in range(NCH):
        t = 0
        while t < NT:
            t2, cols = t, 0
            while t2 < NT and cols + K[k, t2] <= BUFC:
                cols += int(K[k, t2]); t2 += 1
            assert t2 > t
            buffers.append((k, t, t2, cols))
            t = t2

    # runs: consecutive tiles in one buffer with equal K, <= RUNC cols
    runs = []      # (k, buf_idx, t_lo, t_hi, local_col0)
    for bi, (k, tl, th, cols) in enumerate(buffers):
        lc = 0
        t = tl
        while t < th:
            kk = K[k, t]
            t2 = t
            cc = 0
            while t2 < th and K[k, t2] == kk and cc + kk <= RUNC:
                cc += int(kk); t2 += 1
            runs.append((k, bi, t, t2, lc))
            lc += cc
            t = t2

    # per-core slot index arrays (chunk offsets) and padm
    per_core = []
    for c in range(NCORES):
        idx_cols = [np.zeros((int(ncols_k[k]), 128), dtype=np.int64)
                    for k in range(NCH)]
        valid = [np.zeros((int(ncols_k[k]), 128), dtype=bool)
                 for k in range(NCH)]
        for k in range(NCH):
            m = (core == c) & (chunk == k)
            d = dloc[m]
            sv = src_gpos[m] - k * CH
            slot = poslk[c, k, d]
            o = np.argsort(slot, kind="stable")
            se = slot[o]; svs = sv[o]
            cnt = np.bincount(se, minlength=PT)
            st = np.zeros(PT + 1, dtype=np.int64)
            np.cumsum(cnt, out=st[1:])
            colj = np.arange(len(se)) - st[se]
            tl = se // 128
            part = se % 128
            cg = col0[k, tl] + colj
            idx_cols[k][cg, part] = svs
            valid[k][cg, part] = True

        # padm per chunk, concatenated
        padm = np.concatenate(
            [np.where(valid[k], 0.0, -1e30).astype(BF16).T for k in range(NCH)],
            axis=1)  # [128, sum ncols_k]

        # wrapped slot-idx stream, per gather call (per buffer, <= NIMAX)
        widx = []
        for (k, tl, th, cols) in buffers:
            base = int(col0[k, tl])
            arr = idx_cols[k][base : base + cols]           # [cols, 128]
            flat = arr.reshape(-1)                          # col-major slots
            for off in range(0, cols, NIMAX // 128):
                ncc = min(NIMAX // 128, cols - off)
                widx.append(_wrap_idx(flat[off * 128 : (off + ncc) * 128]))

        # al_dst gather idx (rows of own t1shard/t2shard, common positions)
        pad_common = np.arange(ND, PT)
        for k in range(NCH):
            ai = np.zeros(PT, dtype=np.int64)
            real = permk[c, k] >= 0
            ai[real] = gposl[c, permk[c, k, real]]
            ai[~real] = pad_common
            for off in range(0, PT, NIMAX):
                nn = min(NIMAX, PT - off)
                widx.append(_wrap_idx(ai[off : off + nn]))

        # combine gather idx (rows of acc_k in chunk order, by common pos)
        for k in range(NCH):
            ci = np.zeros(PT, dtype=np.int64)
            ci[:ND] = poslk[c, k, perm[c, :ND]]
            ci[ND:] = np.nonzero(permk[c, k] < 0)[0][: PT - ND]
            for off in range(0, PT, NIMAX):
                nn = min(NIMAX, PT - off)
                widx.append(_wrap_idx(ci[off : off + nn]))

        gidx = np.concatenate(widx, axis=1)
        per_core.append(dict(gidx=gidx, padm=np.ascontiguousarray(padm)))

    sched = dict(K=K, col0=col0, ncols_k=ncols_k, buffers=buffers, runs=runs,
                 gidx_w=per_core[0]["gidx"].shape[1],
                 padm_w=per_core[0]["padm"].shape[1])
    return per_core, sched, perm, gpos


_BUILD_CACHE = {}


def _build(sched, phases="all"):
    import os
    import concourse.bass as bass
    import concourse.bacc as bacc
    import concourse.mybir as mybir
    import concourse.tile as tile
    from concourse.masks import make_identity

    use_prelu = USE_PRELU and os.environ.get("GAT_PRELU", "1") == "1"

    f32 = mybir.dt.float32
    bf16 = mybir.dt.bfloat16
    i16 = mybir.dt.int16
    AX = mybir.AxisListType.X
    OP = mybir.AluOpType
    ACT = mybir.ActivationFunctionType

    K = sched["K"]; col0 = sched["col0"]; ncols_k = sched["ncols_k"]
    buffers = sched["buffers"]; runs = sched["runs"]
    GIDXW = sched["gidx_w"]; PADMW = sched["padm_w"]

    scratch = int(os.environ.get("GAT_SCRATCH", "32768"))
    assert NIMAX <= scratch // 16, (NIMAX, scratch)
    nc = bacc.Bacc("TRN2", target_bir_lowering=False, debug=False,
                   num_devices=NCORES, num_swdge_queues=4,
                   dynamic_dma_scratch_size=scratch)

    xts = nc.dram_tensor("xts", [128, PT], bf16, kind="ExternalInput")
    w1e = nc.dram_tensor("w1e", [IN, 80], bf16, kind="ExternalInput")
    b1e = nc.dram_tensor("b1e", [128, 80], f32, kind="ExternalInput")
    w2e = nc.dram_tensor("w2e", [HID, 4], bf16, kind="ExternalInput")
    b2e = nc.dram_tensor("b2e", [128, 4], bf16, kind="ExternalInput")
    gidx = nc.dram_tensor("gidx", [128, GIDXW], i16, kind="ExternalInput")
    padm = nc.dram_tensor("padm", [128, PADMW], bf16, kind="ExternalInput")

    t1shard = nc.dram_tensor("t1shard", [PT, EW], bf16, kind="Internal")
    table1 = nc.dram_tensor("table1", [NPAD, EW], bf16, kind="Internal",
                            addr_space="Shared")
    acc1 = nc.dram_tensor("acc1", [NCH * PT, EW], bf16, kind="Internal")
    t2shard = nc.dram_tensor("t2shard", [PT, EW], bf16, kind="Internal")
    table2 = nc.dram_tensor("table2", [NPAD, EW], bf16, kind="Internal",
                            addr_space="Shared")
    acc2 = nc.dram_tensor("acc2", [NCH * PT, EW], bf16, kind="Internal")
    outp = nc.dram_tensor("outp", [PT, OUT], f32, kind="ExternalOutput")

    # stream cursors (host-side bookkeeping of gidx layout)
    # layout: [slot-call wraps (buffer order)] [adt wraps x4] [cmb wraps x4]
    slot_call_off = []   # per buffer: list of (width_off, ncc)
    woff = 0
    for (k, tl, th, cols) in buffers:
        calls = []
        for off in range(0, cols, NIMAX // 128):
            ncc = min(NIMAX // 128, cols - off)
            calls.append((woff, ncc))
            woff += ncc * 8
        slot_call_off.append(calls)
    adt_call_off = []
    for k in range(NCH):
        calls = []
        for off in range(0, PT, NIMAX):
            nn = min(NIMAX, PT - off)
            calls.append((woff, nn))
            woff += nn // 16
        adt_call_off.append(calls)
    cmb_call_off = []
    for k in range(NCH):
        calls = []
        for off in range(0, PT, NIMAX):
            nn = min(NIMAX, PT - off)
            calls.append((woff, nn))
            woff += nn // 16
        cmb_call_off.append(calls)
    assert woff == GIDXW, (woff, GIDXW)

    padm_off = np.concatenate([[0], np.cumsum(ncols_k)])

    gq = [0]

    def nextq():
        q = gq[0] % 4
        gq[0] += 1
        return q

    with tile.TileContext(nc) as tc:
        with (
            tc.tile_pool(name="const", bufs=1) as cpool,
            tc.tile_pool(name="gath", bufs=2) as gpool,
            tc.tile_pool(name="work", bufs=2) as wpool,
            tc.tile_pool(name="acc", bufs=1) as apool,
            tc.tile_pool(name="cmb", bufs=2) as mpool,
            tc.tile_pool(name="ps", bufs=3, space="PSUM") as pspool,
            tc.tile_pool(name="ps2", bufs=2, space="PSUM") as ps2pool,
        ):
            ident = cpool.tile([128, 128], bf16)
            make_identity(nc, ident[:])
            w1es = cpool.tile([IN, 80], bf16)
            nc.sync.dma_start(out=w1es[:], in_=w1e[:])
            b1es = cpool.tile([128, 80], f32)
            nc.sync.dma_start(out=b1es[:], in_=b1e[:])
            w2es = cpool.tile([HID, 4], bf16)
            nc.sync.dma_start(out=w2es[:], in_=w2e[:])
            b2es = cpool.tile([128, 4], bf16)
            nc.sync.dma_start(out=b2es[:], in_=b2e[:])

            # ---- node phase (sharded): t1all = [al_src | h+b1 | al_dst]
            xtile = gpool.tile([128, PT], bf16, tag="V", bufs=2)
            nc.sync.dma_start(out=xtile[:], in_=xts[:])
            t1all = cpool.tile([128, NT, 80], bf16)
            for b in range(NT):
                g1 = pspool.tile([128, 80], f32, tag="g1")
                nc.tensor.matmul(out=g1[:], lhsT=xtile[:, b * 128 : (b + 1) * 128],
                                 rhs=w1es[:], start=True, stop=True)
                nc.vector.tensor_tensor(out=t1all[:, b, :], in0=g1[:],
                                        in1=b1es[:], op=OP.add)
            dst1 = t1shard[:, 0:80].rearrange("(b p) v -> p b v", p=128)
            nc.sync.dma_start(out=dst1, in_=t1all[:])
            nc.gpsimd.collective_compute(
                "AllGather", OP.bypass,
                replica_groups=[list(range(NCORES))],
                ins=[t1shard[:]], outs=[table1[:]],
            )

            def leaky_exp(eT, exT, cols, nh, tagsfx):
                """exp(leaky_relu(eT)) -> exT (bf16), via ACT engine."""
                if use_prelu:
                    lk = wpool.tile([128, RUNC, 8], f32, tag="lk")
                    nc.scalar.activation(out=lk[:, 0:cols, 0:nh],
                                         in_=eT[:, 0:cols, 0:nh],
                                         func=ACT.Prelu, alpha=NEG)
                    nc.scalar.activation(out=exT[:, 0:cols, 0:nh],
                                         in_=lk[:, 0:cols, 0:nh], func=ACT.Exp)
                else:
                    lk = wpool.tile([128, RUNC, 8], f32, tag="lk")
                    nc.vector.tensor_scalar(out=lk[:, 0:cols, 0:nh],
                                            in0=eT[:, 0:cols, 0:nh],
                                            scalar1=NEG, scalar2=None,
                                            op0=OP.mult)
                    nc.vector.tensor_tensor(out=lk[:, 0:cols, 0:nh],
                                            in0=lk[:, 0:cols, 0:nh],
                                            in1=eT[:, 0:cols, 0:nh], op=OP.max)
                    nc.scalar.activation(out=exT[:, 0:cols, 0:nh],
                                         in_=lk[:, 0:cols, 0:nh], func=ACT.Exp)

            # ---- chunk edge pass (layer = 1 or 2)
            def edge_pass(layer):
                shard = t1shard if layer == 1 else t2shard
                tabT = table1 if layer == 1 else table2
                accT = acc1 if layer == 1 else acc2
                NH = 8 if layer == 1 else 1
                NV = HID if layer == 1 else OUT
                vlo = NH            # value cols start (after al_src cols)
                alo = 72 if layer == 1 else 3   # al_dst col in shard row
                AW = NH + NV        # acc row width

                for k in range(NCH):
                    # al_dst for this chunk's dst ordering
                    adtf = gpool.tile([128, NT, EW], bf16, tag="V", bufs=2)
                    for ci, (wo, nn) in enumerate(adt_call_off[k]):
                        it = wpool.tile([128, IXW], i16, tag="ix")
                        nc.sync.dma_start(out=it[:, 0 : nn // 16],
                                          in_=gidx[:, wo : wo + nn // 16])
                        off = ci * (NIMAX // 128)
                        nc.gpsimd.dma_gather(
                            adtf[:, off : off + nn // 128, :], shard[:],
                            it[:, 0 : nn // 16], nn, nn, EW,
                            queue_num=nextq())
                    adt = wpool.tile([128, NT, NH], f32, tag="adt")
                    nc.vector.tensor_copy(out=adt[:],
                                          in_=adtf[:, :, alo : alo + NH])

                    accs = apool.tile([128, NT, AW], bf16, tag="accs")
                    pmt = wpool.tile([128, int(ncols_k[k])], bf16, tag="pm")
                    po = int(padm_off[k])
                    nc.scalar.dma_start(out=pmt[:],
                                        in_=padm[:, po : po + int(ncols_k[k])])

                    for bi, (bk, tl, th, cols) in enumerate(buffers):
                        if bk != k:
                            continue
                        Vb = gpool.tile([128, BUFC, EW], bf16, tag="V", bufs=2)
                        for (wo, ncc) in slot_call_off[bi]:
                            it = wpool.tile([128, IXW], i16, tag="ix")
                            nc.scalar.dma_start(out=it[:, 0 : ncc * 8],
                                                in_=gidx[:, wo : wo + ncc * 8])
                            co = (wo - slot_call_off[bi][0][0]) // 8
                            nc.gpsimd.dma_gather(
                                Vb[:, co : co + ncc, :],
                                tabT[k * CH : (k + 1) * CH, :],
                                it[:, 0 : ncc * 8], ncc * 128, ncc * 128, EW,
                                queue_num=nextq())

                        for (rk, rbi, rtl, rth, lc) in runs:
                            if rbi != bi:
                                continue
                            nt = rth - rtl
                            kt = int(K[k, rtl])
                            cc = nt * kt
                            # e = al_src + al_dst + mask
                            eT = wpool.tile([128, RUNC, 8], f32, tag="eT")
                            e4 = eT[:, 0:cc, 0:NH].rearrange(
                                "p (n j) h -> p n j h", n=nt)
                            nc.vector.tensor_tensor(
                                out=e4,
                                in0=Vb[:, lc : lc + cc, 0:NH].rearrange(
                                    "p (n j) h -> p n j h", n=nt),
                                in1=adt[:, rtl:rth, :].unsqueeze(2)
                                    .to_broadcast([128, nt, kt, NH]),
                                op=OP.add)
                            pc0 = po + int(col0[k, rtl])
                            nc.vector.tensor_tensor(
                                out=e4, in0=e4,
                                in1=pmt[:, int(col0[k, rtl]) : int(col0[k, rtl]) + cc]
                                    .rearrange("p (n j) -> p n j", n=nt)
                                    .unsqueeze(3)
                                    .to_broadcast([128, nt, kt, NH]),
                                op=OP.add)
                            exT = wpool.tile([128, RUNC, 8], bf16, tag="ex")
                            leaky_exp(eT, exT, cc, NH, f"{layer}")
                            # weighted values
                            Wf = wpool.tile([128, RUNC, HID], bf16, tag="Wf")
                            if layer == 1:
                                nc.vector.tensor_tensor(
                                    out=Wf[:, 0:cc, :].rearrange(
                                        "p c (h w) -> p c h w", h=NH),
                                    in0=Vb[:, lc : lc + cc, vlo : vlo + NV]
                                        .rearrange("p c (h w) -> p c h w", h=NH),
                                    in1=exT[:, 0:cc, 0:NH].unsqueeze(3)
                                        .to_broadcast([128, cc, NH, C1]),
                                    op=OP.mult)
                            else:
                                nc.vector.tensor_tensor(
                                    out=Wf[:, 0:cc, 0:NV],
                                    in0=Vb[:, lc : lc + cc, vlo : vlo + NV],
                                    in1=exT[:, 0:cc, 0:1]
                                        .to_broadcast([128, cc, NV]),
                                    op=OP.mult)
                            # reduce den / num into acc rows (bf16 sums of
                            # <=24 O(1) terms; validated vs reference)
                            with nc.allow_low_precision(reason="short bf16 "
                                                        "attention sums"):
                                nc.vector.tensor_reduce(
                                    out=accs[:, rtl:rth, 0:NH],
                                    in_=exT[:, 0:cc, 0:NH].rearrange(
                                        "p (n j) h -> p n h j", n=nt),
                                    axis=AX, op=OP.add)
                                nc.vector.tensor_reduce(
                                    out=accs[:, rtl:rth, NH:AW],
                                    in_=Wf[:, 0:cc, 0:NV].rearrange(
                                        "p (n j) f -> p n f j", n=nt),
                                    axis=AX, op=OP.add)

                    dsta = accT[k * PT : (k + 1) * PT, 0:AW].rearrange(
                        "(b p) v -> p b v", p=128)
                    nc.sync.dma_start(out=dsta, in_=accs[:])

            # ---- combine pass (layer 1: -> t2shard+AllGather; 2: -> outp)
            def combine(layer):
                accT = acc1 if layer == 1 else acc2
                NH = 8 if layer == 1 else 1
                NV = HID if layer == 1 else OUT
                AW = NH + NV
                selfT = t1all if layer == 1 else t2all
                alo = 72 if layer == 1 else 3

                tot = apool.tile([128, NT, AW], bf16, tag=f"tot{layer}")
                for k in range(NCH):
                    for ci, (wo, nn) in enumerate(cmb_call_off[k]):
                        it = wpool.tile([128, IXW], i16, tag="ix")
                        nc.sync.dma_start(out=it[:, 0 : nn // 16],
                                          in_=gidx[:, wo : wo + nn // 16])
                        gf = gpool.tile([128, NIMAX // 128, EW], bf16,
                                        tag="V", bufs=2)
                        nc.gpsimd.dma_gather(
                            gf[:, 0 : nn // 128, :],
                            accT[k * PT : (k + 1) * PT, :],
                            it[:, 0 : nn // 16], nn, nn, EW,
                            queue_num=nextq())
                        b0 = ci * (NIMAX // 128)
                        nb = nn // 128
                        if k == 0:
                            nc.vector.tensor_copy(
                                out=tot[:, b0 : b0 + nb, :],
                                in_=gf[:, 0:nb, 0:AW])
                        else:
                            nc.vector.tensor_tensor(
                                out=tot[:, b0 : b0 + nb, :],
                                in0=tot[:, b0 : b0 + nb, :],
                                in1=gf[:, 0:nb, 0:AW], op=OP.add)

                o2 = None
                if layer == 2:
                    o2 = mpool.tile([128, NT, OUT], f32, tag="o2", bufs=1,
                                    name="o2")
                for h0 in range(0, NT, FG):
                    nb = min(FG, NT - h0)
                    sT = selfT[:, h0 : h0 + nb, :]
                    tT = tot[:, h0 : h0 + nb, :]
                    # self loop: e = al_src + al_dst (own row) -> exp
                    eS = mpool.tile([128, FG, 8], f32, tag="eS", bufs=1)
                    nc.vector.tensor_tensor(out=eS[:, 0:nb, 0:NH],
                                            in0=sT[:, :, 0:NH],
                                            in1=sT[:, :, alo : alo + NH],
                                            op=OP.add)
                    lkS = mpool.tile([128, FG, 8], f32, tag="lkS", bufs=1)
                    if use_prelu:
                        nc.scalar.activation(out=lkS[:, 0:nb, 0:NH],
                                             in_=eS[:, 0:nb, 0:NH],
                                             func=ACT.Prelu, alpha=NEG)
                    else:
                        nc.vector.tensor_scalar(out=lkS[:, 0:nb, 0:NH],
                                                in0=eS[:, 0:nb, 0:NH],
                                                scalar1=NEG, scalar2=None,
                                                op0=OP.mult)
                        nc.vector.tensor_tensor(out=lkS[:, 0:nb, 0:NH],
                                                in0=lkS[:, 0:nb, 0:NH],
                                                in1=eS[:, 0:nb, 0:NH],
                                                op=OP.max)
                    exS = mpool.tile([128, FG, 8], bf16, tag="exS", bufs=1)
                    nc.scalar.activation(out=exS[:, 0:nb, 0:NH],
                                         in_=lkS[:, 0:nb, 0:NH], func=ACT.Exp)
                    den = mpool.tile([128, FG, 8], f32, tag="den", bufs=1)
                    nc.vector.tensor_tensor(out=den[:, 0:nb, 0:NH],
                                            in0=tT[:, :, 0:NH],
                                            in1=exS[:, 0:nb, 0:NH], op=OP.add)
                    wS = mpool.tile([128, FG, HID], bf16, tag="wS", bufs=1)
                    if layer == 1:
                        nc.vector.tensor_tensor(
                            out=wS[:, 0:nb, :].rearrange(
                                "p b (h w) -> p b h w", h=NH),
                            in0=sT[:, :, NH : NH + NV].rearrange(
                                "p b (h w) -> p b h w", h=NH),
                            in1=exS[:, 0:nb, 0:NH].unsqueeze(3).to_broadcast(
                                [128, nb, NH, C1]),
                            op=OP.mult)
                    else:
                        nc.vector.tensor_tensor(
                            out=wS[:, 0:nb, 0:NV],
                            in0=sT[:, :, NH : NH + NV],
                            in1=exS[:, 0:nb, 0:1].to_broadcast([128, nb, NV]),
                            op=OP.mult)
                    num = mpool.tile([128, FG, HID], f32, tag="num", bufs=1)
                    nc.vector.tensor_tensor(out=num[:, 0:nb, 0:NV],
                                            in0=tT[:, :, NH:AW],
                                            in1=wS[:, 0:nb, 0:NV], op=OP.add)
                    rden = mpool.tile([128, FG, 8], f32, tag="rden", bufs=1)
                    nc.vector.reciprocal(out=rden[:, 0:nb, 0:NH],
                                         in_=den[:, 0:nb, 0:NH])
                    if layer == 2:
                        nc.vector.tensor_tensor(
                            out=o2[:, h0 : h0 + nb, :],
                            in0=num[:, 0:nb, 0:NV],
                            in1=rden[:, 0:nb, 0:1].to_broadcast(
                                [128, nb, OUT]),
                            op=OP.mult)
                        continue
                    # z = num/den, elu(z) = max(z,0) + exp(min(z,0)) - 1
                    z = mpool.tile([128, FG, HID], f32, tag="z", bufs=1)
                    nc.vector.tensor_tensor(
                        out=z[:, 0:nb, :].rearrange("p b (h w) -> p b h w",
                                                    h=NH),
                        in0=num[:, 0:nb, :].rearrange("p b (h w) -> p b h w",
                                                      h=NH),
                        in1=rden[:, 0:nb, 0:NH].unsqueeze(3).to_broadcast(
                            [128, nb, NH, C1]),
                        op=OP.mult)
                    zm = mpool.tile([128, FG, HID], f32, tag="zm", bufs=1)
                    nc.vector.tensor_scalar(out=zm[:, 0:nb, :],
                                            in0=z[:, 0:nb, :], scalar1=0.0,
                                            scalar2=None, op0=OP.min)
                    ze = mpool.tile([128, FG, HID], f32, tag="ze", bufs=1)
                    nc.scalar.activation(out=ze[:, 0:nb, :],
                                         in_=zm[:, 0:nb, :], func=ACT.Exp)
                    zb = mpool.tile([128, FG, HID], bf16, tag="zb", bufs=1)
                    nc.vector.tensor_scalar(out=zb[:, 0:nb, :],
                                            in0=z[:, 0:nb, :], scalar1=0.0,
                                            scalar2=None, op0=OP.max)
                    nc.vector.tensor_tensor(out=zb[:, 0:nb, :],
                                            in0=zb[:, 0:nb, :],
                                            in1=ze[:, 0:nb, :], op=OP.add)
                    nc.vector.tensor_scalar(out=zb[:, 0:nb, :],
                                            in0=zb[:, 0:nb, :], scalar1=-1.0,
                                            scalar2=None, op0=OP.add)
                    for b in range(nb):
                        zT = ps2pool.tile([HID, 128], bf16, tag="zT")
                        nc.tensor.transpose(out=zT[:], in_=zb[:, b, :],
                                            identity=ident[:])
                        zTs = mpool.tile([HID, 128], bf16, tag="zTs")
                        nc.vector.tensor_copy(out=zTs[:], in_=zT[:])
                        g2 = ps2pool.tile([128, 4], f32, tag="g2")
                        nc.tensor.matmul(out=g2[:], lhsT=zTs[:], rhs=w2es[:],
                                         start=True, stop=True)
                        nc.vector.tensor_tensor(out=t2all[:, h0 + b, :],
                                                in0=g2[:], in1=b2es[:],
                                                op=OP.add)
                if layer == 2:
                    dsto = outp[:].rearrange("(b p) v -> p b v", p=128)
                    nc.sync.dma_start(out=dsto, in_=o2[:])
                    return
                dst2 = t2shard[:, 0:4].rearrange("(b p) v -> p b v", p=128)
                nc.sync.dma_start(out=dst2, in_=t2all[:])
                nc.gpsimd.collective_compute(
                    "AllGather", OP.bypass,
                    replica_groups=[list(range(NCORES))],
                    ins=[t2shard[:]], outs=[table2[:]],
                )

            t2all = cpool.tile([128, NT, 4], bf16)
            if phases in ("e1", "c1", "all"):
                edge_pass(1)
            if phases in ("c1", "all"):
                combine(1)
            if phases == "all":
                edge_pass(2)
                combine(2)
            if phases != "all":
                od = mpool.tile([128, NT, OUT], f32, tag="o2", bufs=1,
                                name="od")
                srcT = t2all if phases == "c1" else t1all
                nc.vector.tensor_copy(out=od[:], in_=srcT[:, :, 0:OUT])
                dsto = outp[:].rearrange("(b p) v -> p b v", p=128)
                nc.sync.dma_start(out=dsto, in_=od[:])

    nc.compile()
    return nc


def kernel(**inputs):
    from concourse.bass_utils import run_bass_kernel_spmd

    x = np.asarray(inputs["x"], dtype=np.float32)
    ei = np.asarray(inputs["edge_index"]).astype(np.int64)
    w1 = np.asarray(inputs["W1"], dtype=np.float32)
    a1s = np.asarray(inputs["a1_src"], dtype=np.float32)
    a1d = np.asarray(inputs["a1_dst"], dtype=np.float32)
    b1 = np.asarray(inputs["b1"], dtype=np.float32)
    w2 = np.asarray(inputs["W2"], dtype=np.float32)
    a2s = np.asarray(inputs["a2_src"], dtype=np.float32)
    a2d = np.asarray(inputs["a2_dst"], dtype=np.float32)
    b2 = np.asarray(inputs["b2"], dtype=np.float32)

    src, dst = ei[0], ei[1]
    per_core, sched, perm, gpos = _plan(src, dst)

    # weights: w1e = [W1@A1s | W1 | W1@A1d]  (al_src | h | al_dst columns)
    A1s = np.zeros((HID, H1), dtype=np.float32)
    A1d = np.zeros((HID, H1), dtype=np.float32)
    for h in range(H1):
        A1s[h * C1 : (h + 1) * C1, h] = a1s[h]
        A1d[h * C1 : (h + 1) * C1, h] = a1d[h]
    w1e = np.concatenate([w1 @ A1s, w1, w1 @ A1d], axis=1)      # [128, 80]
    w2e = np.concatenate([w2 @ a2s.T, w2, w2 @ a2d.T], axis=1)  # [64, 4]
    b1e = np.zeros((128, 80), dtype=np.float32)
    b1e[:, 8 : 8 + HID] = b1[None, :]
    b2e = np.zeros((128, 4), dtype=np.float32)
    b2e[:, 1 : 1 + OUT] = b2[None, :]

    import os
    phases = os.environ.get("GAT_PHASES", "all")
    key = (tuple(sched["K"].reshape(-1).tolist()), phases)
    if key not in _BUILD_CACHE:
        _BUILD_CACHE[key] = _build(sched, phases)
    nc = _BUILD_CACHE[key]

    common = dict(w1e=w1e.astype(BF16), b1e=b1e, w2e=w2e.astype(BF16),
                  b2e=b2e.astype(BF16))
    in_maps = []
    for c in range(NCORES):
        xp_c = np.zeros((PT, IN), dtype=np.float32)
        real = perm[c] >= 0
        xp_c[real] = x[c * ND + perm[c][real]]
        m = dict(common)
        m["xts"] = np.ascontiguousarray(xp_c.T).astype(BF16)
        m["gidx"] = per_core[c]["gidx"]
        m["padm"] = per_core[c]["padm"]
        in_maps.append(m)

    global _LAST_IN_MAPS
    _LAST_IN_MAPS = in_maps
    res = run_bass_kernel_spmd(nc, in_maps, list(range(NCORES)))

    out = np.zeros((N, OUT), dtype=np.float32)
    for c in range(NCORES):
        op = res.results[c]["outp"]       # [PT, 2] in common (gpos) order
        real = perm[c] >= 0
        out[c * ND + perm[c][real]] = op[real]
    return out


# revision 13
# speedup vs baseline: 2.2775x; 2.2775x over previous
"""GAT (2-layer, PyG-style) on 8 Trainium2 NeuronCores via Bass/Tile.

v2 strategy (chunked 1-node-row gathers, no on-chip unpacking):
- Nodes sharded by dst across 8 cores; per core dsts are degree-sorted into
  98 tiles of 128 (partition = dst). Node features live in tables whose row
  order is this global degree-sorted position ("gpos"); x is permuted to
  gpos order host-side and pre-transposed, so the node phase is a straight
  matmul and the self-loop rows are contiguous.
- dma_gather rows must be 256B multiples and int16-indexed (<=32768 rows),
  so the edge phase runs as 4 passes over src-gpos chunks of 25088 rows,
  each gathering 1-node 256B rows directly (no 4-pack + one-hot select).
  Per-chunk degree-sorted dst tiling keeps slot padding at ~2.6%.
- Each chunk pass accumulates partial [den|num] per dst in its own dst
  order; a combine phase gathers the 4 partials back to common order, adds
  the self-loop term, and finalizes (softmax divide, elu, layer-2 linear).
- Layer 2 re-uses the exact same index streams (same edges, same slots)
  against a [NPAD, 256B] table of (al2_src, y0, y1, al2_dst) rows.
- Node phase is sharded (each core computes its 12544 rows) and shards are
  exchanged with AllGather.
"""

import numpy as np
import ml_dtypes

BF16 = ml_dtypes.bfloat16

N = 100_000
E = 3_200_000
IN = 128
H1, C1 = 8, 8
HID = H1 * C1          # 64
OUT = 2
NEG = 0.2
NCORES = 8
ND = N // NCORES       # real dsts per core: 12500
NT = 98                # tiles per core (98*128 = 12544)
PT = NT * 128          # padded dst slots per core
NPAD = PT * NCORES     # 100352
NCH = 4                # src chunks
CH = NPAD // NCH       # 25088 rows per chunk (int16-addressable)
EW = 128               # table row elems (bf16) = 256B
import os as _os
NIMAX = int(_os.environ.get("GAT_NIMAX", "2048"))  # idx per dma_gather call
BUFC = 96              # gather-buffer columns (tile-aligned packing limit)
RUNC = 64              # max columns per compute run
FG = 25                # finalize group cols
IXW = 784              # idx tile width (int16 cols)
USE_PRELU = True


def _wrap_idx(flat):
    """int16 index array -> [128, n/16] wrapped-in-16-partitions, replicated x8."""
    n = flat.shape[0]
    assert n % 16 == 0
    w = flat.reshape(n // 16, 16).T            # [16, n/16]
    return np.tile(w, (8, 1)).astype(np.int16)  # [128, n/16]


def _plan(src, dst):
    """Host-side planning. Returns per-core streams + the common schedule."""
    core = dst // ND
    dloc = dst % ND

    # common (total-degree) order per core -> gpos
    deg_tot = np.zeros((NCORES, ND), dtype=np.int64)
    for c in range(NCORES):
        deg_tot[c] = np.bincount(dloc[core == c], minlength=ND)
    perm = np.full((NCORES, PT), -1, dtype=np.int64)
    gposl = np.zeros((NCORES, ND), dtype=np.int64)  # dloc -> common position
    for c in range(NCORES):
        order = np.argsort(-deg_tot[c], kind="stable")
        perm[c, :ND] = order
        gposl[c, order] = np.arange(ND)
    gpos = np.zeros(N, dtype=np.int64)
    for c in range(NCORES):
        gpos[c * ND + perm[c, :ND]] = c * PT + np.arange(ND)

    src_gpos = gpos[src]
    chunk = src_gpos // CH

    # per-(core, chunk) degree-sorted slot assignment
    permk = np.full((NCORES, NCH, PT), -1, dtype=np.int64)
    poslk = np.zeros((NCORES, NCH, ND), dtype=np.int64)
    degk_all = np.zeros((NCORES, NCH, ND), dtype=np.int64)
    for c in range(NCORES):
        for k in range(NCH):
            m = (core == c) & (chunk == k)
            degk = np.bincount(dloc[m], minlength=ND)
            degk_all[c, k] = degk
            ok = np.argsort(-degk, kind="stable")
            permk[c, k, :ND] = ok
            poslk[c, k, ok] = np.arange(ND)

    # common K schedule: K[k, t] = max in-chunk degree over cores in tile t
    K = np.zeros((NCH, NT), dtype=np.int64)
    for k in range(NCH):
        for t in range(NT):
            mx = 1
            for c in range(NCORES):
                sl = permk[c, k, t * 128 : (t + 1) * 128]
                real = sl[sl >= 0]
                if real.size:
                    mx = max(mx, int(degk_all[c, k, real].max()))
            K[k, t] = mx
    col0 = np.zeros((NCH, NT), dtype=np.int64)
    for k in range(NCH):
        col0[k] = np.cumsum(K[k]) - K[k]
    ncols_k = K.sum(axis=1)

    # buffer packing (tile-aligned, <= BUFC cols) and gather-call layout
    buffers = []   # (k, t_lo, t_hi, cols)
    for k in range(NCH):
        t = 0
        while t < NT:
            t2, cols = t, 0
            while t2 < NT and cols + K[k, t2] <= BUFC:
                cols += int(K[k, t2]); t2 += 1
            assert t2 > t
            buffers.append((k, t, t2, cols))
            t = t2

    # runs: consecutive tiles in one buffer with equal K, <= RUNC cols
    runs = []      # (k, buf_idx, t_lo, t_hi, local_col0)
    for bi, (k, tl, th, cols) in enumerate(buffers):
        lc = 0
        t = tl
        while t < th:
            kk = K[k, t]
            t2 = t
            cc = 0
            while t2 < th and K[k, t2] == kk and cc + kk <= RUNC:
                cc += int(kk); t2 += 1
            runs.append((k, bi, t, t2, lc))
            lc += cc
            t = t2

    # per-core slot index arrays (chunk offsets) and padm
    per_core = []
    for c in range(NCORES):
        idx_cols = [np.zeros((int(ncols_k[k]), 128), dtype=np.int64)
                    for k in range(NCH)]
        valid = [np.zeros((int(ncols_k[k]), 128), dtype=bool)
                 for k in range(NCH)]
        for k in range(NCH):
            m = (core == c) & (chunk == k)
            d = dloc[m]
            sv = src_gpos[m] - k * CH
            slot = poslk[c, k, d]
            o = np.argsort(slot, kind="stable")
            se = slot[o]; svs = sv[o]
            cnt = np.bincount(se, minlength=PT)
            st = np.zeros(PT + 1, dtype=np.int64)
            np.cumsum(cnt, out=st[1:])
            colj = np.arange(len(se)) - st[se]
            tl = se // 128
            part = se % 128
            cg = col0[k, tl] + colj
            idx_cols[k][cg, part] = svs
            valid[k][cg, part] = True

        # padm per chunk, concatenated
        padm = np.concatenate(
            [np.where(valid[k], 0.0, -1e30).astype(BF16).T for k in range(NCH)],
            axis=1)  # [128, sum ncols_k]

        # wrapped slot-idx stream, per gather call (per buffer, <= NIMAX)
        widx = []
        for (k, tl, th, cols) in buffers:
            base = int(col0[k, tl])
            arr = idx_cols[k][base : base + cols]           # [cols, 128]
            flat = arr.reshape(-1)                          # col-major slots
            for off in range(0, cols, NIMAX // 128):
                ncc = min(NIMAX // 128, cols - off)
                widx.append(_wrap_idx(flat[off * 128 : (off + ncc) * 128]))

        # al_dst gather idx (rows of own t1shard/t2shard, common positions)
        pad_common = np.arange(ND, PT)
        for k in range(NCH):
            ai = np.zeros(PT, dtype=np.int64)
            real = permk[c, k] >= 0
            ai[real] = gposl[c, permk[c, k, real]]
            ai[~real] = pad_common
            for off in range(0, PT, NIMAX):
                nn = min(NIMAX, PT - off)
                widx.append(_wrap_idx(ai[off : off + nn]))

        # combine gather idx (rows of acc_k in chunk order, by common pos)
        for k in range(NCH):
            ci = np.zeros(PT, dtype=np.int64)
            ci[:ND] = poslk[c, k, perm[c, :ND]]
            ci[ND:] = np.nonzero(permk[c, k] < 0)[0][: PT - ND]
            for off in range(0, PT, NIMAX):
                nn = min(NIMAX, PT - off)
                widx.append(_wrap_idx(ci[off : off + nn]))

        gidx = np.concatenate(widx, axis=1)
        per_core.append(dict(gidx=gidx, padm=np.ascontiguousarray(padm)))

    sched = dict(K=K, col0=col0, ncols_k=ncols_k, buffers=buffers, runs=runs,
                 gidx_w=per_core[0]["gidx"].shape[1],
                 padm_w=per_core[0]["padm"].shape[1])
    return per_core, sched, perm, gpos


_BUILD_CACHE = {}


def _build(sched, phases="all"):
    import os
    import concourse.bass as bass
    import concourse.bacc as bacc
    import concourse.mybir as mybir
    import concourse.tile as tile
    from concourse.masks import make_identity

    use_prelu = USE_PRELU and os.environ.get("GAT_PRELU", "1") == "1"

    f32 = mybir.dt.float32
    bf16 = mybir.dt.bfloat16
    i16 = mybir.dt.int16
    AX = mybir.AxisListType.X
    OP = mybir.AluOpType
    ACT = mybir.ActivationFunctionType

    K = sched["K"]; col0 = sched["col0"]; ncols_k = sched["ncols_k"]
    buffers = sched["buffers"]; runs = sched["runs"]
    GIDXW = sched["gidx_w"]; PADMW = sched["padm_w"]

    scratch = int(os.environ.get("GAT_SCRATCH", "32768"))
    assert NIMAX <= scratch // 16, (NIMAX, scratch)
    nc = bacc.Bacc("TRN2", target_bir_lowering=False, debug=False,
                   num_devices=NCORES, num_swdge_queues=4,
                   dynamic_dma_scratch_size=scratch)

    xts = nc.dram_tensor("xts", [128, PT], bf16, kind="ExternalInput")
    w1e = nc.dram_tensor("w1e", [IN, 80], bf16, kind="ExternalInput")
    b1e = nc.dram_tensor("b1e", [128, 80], f32, kind="ExternalInput")
    w2e = nc.dram_tensor("w2e", [HID, 4], bf16, kind="ExternalInput")
    b2e = nc.dram_tensor("b2e", [128, 4], bf16, kind="ExternalInput")
    gidx = nc.dram_tensor("gidx", [128, GIDXW], i16, kind="ExternalInput")
    padm = nc.dram_tensor("padm", [128, PADMW], bf16, kind="ExternalInput")

    t1shard = nc.dram_tensor("t1shard", [PT, EW], bf16, kind="Internal")
    table1 = nc.dram_tensor("table1", [NPAD, EW], bf16, kind="Internal",
                            addr_space="Shared")
    acc1 = nc.dram_tensor("acc1", [NCH * PT, EW], bf16, kind="Internal")
    t2shard = nc.dram_tensor("t2shard", [PT, EW], bf16, kind="Internal")
    table2 = nc.dram_tensor("table2", [NPAD, EW], bf16, kind="Internal",
                            addr_space="Shared")
    acc2 = nc.dram_tensor("acc2", [NCH * PT, EW], bf16, kind="Internal")
    outp = nc.dram_tensor("outp", [128, NT * OUT], f32,
                          kind="ExternalOutput")

    # stream cursors (host-side bookkeeping of gidx layout)
    # layout: [slot-call wraps (buffer order)] [adt wraps x4] [cmb wraps x4]
    slot_call_off = []   # per buffer: list of (width_off, ncc)
    woff = 0
    for (k, tl, th, cols) in buffers:
        calls = []
        for off in range(0, cols, NIMAX // 128):
            ncc = min(NIMAX // 128, cols - off)
            calls.append((woff, ncc))
            woff += ncc * 8
        slot_call_off.append(calls)
    adt_call_off = []
    for k in range(NCH):
        calls = []
        for off in range(0, PT, NIMAX):
            nn = min(NIMAX, PT - off)
            calls.append((woff, nn))
            woff += nn // 16
        adt_call_off.append(calls)
    cmb_call_off = []
    for k in range(NCH):
        calls = []
        for off in range(0, PT, NIMAX):
            nn = min(NIMAX, PT - off)
            calls.append((woff, nn))
            woff += nn // 16
        cmb_call_off.append(calls)
    assert woff == GIDXW, (woff, GIDXW)

    padm_off = np.concatenate([[0], np.cumsum(ncols_k)])

    gq = [0]

    def nextq():
        q = gq[0] % 4
        gq[0] += 1
        return q

    with tile.TileContext(nc) as tc:
        with (
            tc.tile_pool(name="const", bufs=1) as cpool,
            tc.tile_pool(name="gath", bufs=2) as gpool,
            tc.tile_pool(name="work", bufs=2) as wpool,
            tc.tile_pool(name="acc", bufs=1) as apool,
            tc.tile_pool(name="cmb", bufs=2) as mpool,
            tc.tile_pool(name="ps", bufs=3, space="PSUM") as pspool,
            tc.tile_pool(name="ps2", bufs=2, space="PSUM") as ps2pool,
        ):
            ident = cpool.tile([128, 128], bf16)
            make_identity(nc, ident[:])
            w1es = cpool.tile([IN, 80], bf16)
            nc.sync.dma_start(out=w1es[:], in_=w1e[:])
            b1es = cpool.tile([128, 80], f32)
            nc.sync.dma_start(out=b1es[:], in_=b1e[:])
            w2es = cpool.tile([HID, 4], bf16)
            nc.sync.dma_start(out=w2es[:], in_=w2e[:])
            b2es = cpool.tile([128, 4], bf16)
            nc.sync.dma_start(out=b2es[:], in_=b2e[:])

            # ---- node phase (sharded): t1all = [al_src | h+b1 | al_dst]
            xtile = gpool.tile([128, PT], bf16, tag="V", bufs=2)
            nc.sync.dma_start(out=xtile[:], in_=xts[:])
            t1all = cpool.tile([128, NT, 80], bf16)
            for b in range(NT):
                g1 = pspool.tile([128, 80], f32, tag="g1")
                nc.tensor.matmul(out=g1[:], lhsT=xtile[:, b * 128 : (b + 1) * 128],
                                 rhs=w1es[:], start=True, stop=True)
                nc.vector.tensor_tensor(out=t1all[:, b, :], in0=g1[:],
                                        in1=b1es[:], op=OP.add)
            dst1 = t1shard[:, 0:80].rearrange("(b p) v -> p b v", p=128)
            nc.sync.dma_start(out=dst1, in_=t1all[:])
            nc.gpsimd.collective_compute(
                "AllGather", OP.bypass,
                replica_groups=[list(range(NCORES))],
                ins=[t1shard[:]], outs=[table1[:]],
            )

            def leaky_exp(eT, exT, cols, nh, tagsfx):
                """exp(leaky_relu(eT)) -> exT (bf16), via ACT engine."""
                if use_prelu:
                    lk = wpool.tile([128, RUNC, 8], f32, tag="lk")
                    nc.scalar.activation(out=lk[:, 0:cols, 0:nh],
                                         in_=eT[:, 0:cols, 0:nh],
                                         func=ACT.Prelu, alpha=NEG)
                    nc.scalar.activation(out=exT[:, 0:cols, 0:nh],
                                         in_=lk[:, 0:cols, 0:nh], func=ACT.Exp)
                else:
                    lk = wpool.tile([128, RUNC, 8], f32, tag="lk")
                    nc.vector.tensor_scalar(out=lk[:, 0:cols, 0:nh],
                                            in0=eT[:, 0:cols, 0:nh],
                                            scalar1=NEG, scalar2=None,
                                            op0=OP.mult)
                    nc.vector.tensor_tensor(out=lk[:, 0:cols, 0:nh],
                                            in0=lk[:, 0:cols, 0:nh],
                                            in1=eT[:, 0:cols, 0:nh], op=OP.max)
                    nc.scalar.activation(out=exT[:, 0:cols, 0:nh],
                                         in_=lk[:, 0:cols, 0:nh], func=ACT.Exp)

            # ---- chunk edge pass (layer = 1 or 2)
            def edge_pass(layer):
                shard = t1shard if layer == 1 else t2shard
                tabT = table1 if layer == 1 else table2
                accT = acc1 if layer == 1 else acc2
                NH = 8 if layer == 1 else 1
                NV = HID if layer == 1 else OUT
                vlo = NH            # value cols start (after al_src cols)
                alo = 72 if layer == 1 else 3   # al_dst col in shard row
                AW = NH + NV        # acc row width

                for k in range(NCH):
                    # al_dst for this chunk's dst ordering
                    adtf = gpool.tile([128, NT, EW], bf16, tag="V", bufs=2)
                    aw0 = adt_call_off[k][0][0]
                    awn = sum(nn for (_, nn) in adt_call_off[k]) // 16
                    ait = wpool.tile([128, IXW], i16, tag="ix")
                    nc.sync.dma_start(out=ait[:, 0:awn],
                                      in_=gidx[:, aw0 : aw0 + awn])
                    for ci, (wo, nn) in enumerate(adt_call_off[k]):
                        off = ci * (NIMAX // 128)
                        lo = (wo - aw0)
                        nc.gpsimd.dma_gather(
                            adtf[:, off : off + nn // 128, :], shard[:],
                            ait[:, lo : lo + nn // 16], nn, nn, EW,
                            queue_num=nextq())
                    adt = wpool.tile([128, NT, NH], f32, tag="adt")
                    nc.vector.tensor_copy(out=adt[:],
                                          in_=adtf[:, :, alo : alo + NH])

                    accs = apool.tile([128, NT, AW], bf16, tag="accs",
                                      bufs=2)
                    pmt = wpool.tile([128, int(ncols_k[k])], bf16, tag="pm")
                    po = int(padm_off[k])
                    nc.scalar.dma_start(out=pmt[:],
                                        in_=padm[:, po : po + int(ncols_k[k])])

                    for bi, (bk, tl, th, cols) in enumerate(buffers):
                        if bk != k:
                            continue
                        Vb = gpool.tile([128, BUFC, EW], bf16, tag="V", bufs=2)
                        bw0 = slot_call_off[bi][0][0]
                        bwn = sum(ncc for (_, ncc) in slot_call_off[bi]) * 8
                        it = wpool.tile([128, IXW], i16, tag="ix")
                        nc.scalar.dma_start(out=it[:, 0:bwn],
                                            in_=gidx[:, bw0 : bw0 + bwn])
                        for (wo, ncc) in slot_call_off[bi]:
                            co = (wo - bw0) // 8
                            nc.gpsimd.dma_gather(
                                Vb[:, co : co + ncc, :],
                                tabT[k * CH : (k + 1) * CH, :],
                                it[:, co * 8 : (co + ncc) * 8],
                                ncc * 128, ncc * 128, EW,
                                queue_num=nextq())

                        for (rk, rbi, rtl, rth, lc) in runs:
                            if rbi != bi:
                                continue
                            nt = rth - rtl
                            kt = int(K[k, rtl])
                            cc = nt * kt
                            # e = al_src + al_dst + mask
                            eT = wpool.tile([128, RUNC, 8], f32, tag="eT")
                            e4 = eT[:, 0:cc, 0:NH].rearrange(
                                "p (n j) h -> p n j h", n=nt)
                            nc.vector.tensor_tensor(
                                out=e4,
                                in0=Vb[:, lc : lc + cc, 0:NH].rearrange(
                                    "p (n j) h -> p n j h", n=nt),
                                in1=adt[:, rtl:rth, :].unsqueeze(2)
                                    .to_broadcast([128, nt, kt, NH]),
                                op=OP.add)
                            pc0 = po + int(col0[k, rtl])
                            nc.vector.tensor_tensor(
                                out=e4, in0=e4,
                                in1=pmt[:, int(col0[k, rtl]) : int(col0[k, rtl]) + cc]
                                    .rearrange("p (n j) -> p n j", n=nt)
                                    .unsqueeze(3)
                                    .to_broadcast([128, nt, kt, NH]),
                                op=OP.add)
                            exT = wpool.tile([128, RUNC, 8], bf16, tag="ex")
                            leaky_exp(eT, exT, cc, NH, f"{layer}")
                            # weighted values
                            Wf = wpool.tile([128, RUNC, HID], bf16, tag="Wf")
                            if layer == 1:
                                nc.vector.tensor_tensor(
                                    out=Wf[:, 0:cc, :].rearrange(
                                        "p c (h w) -> p c h w", h=NH),
                                    in0=Vb[:, lc : lc + cc, vlo : vlo + NV]
                                        .rearrange("p c (h w) -> p c h w", h=NH),
                                    in1=exT[:, 0:cc, 0:NH].unsqueeze(3)
                                        .to_broadcast([128, cc, NH, C1]),
                                    op=OP.mult)
                            else:
                                nc.vector.tensor_tensor(
                                    out=Wf[:, 0:cc, 0:NV],
                                    in0=Vb[:, lc : lc + cc, vlo : vlo + NV],
                                    in1=exT[:, 0:cc, 0:1]
                                        .to_broadcast([128, cc, NV]),
                                    op=OP.mult)
                            # reduce den / num into acc rows (bf16 sums of
                            # <=24 O(1) terms; validated vs reference)
                            with nc.allow_low_precision(reason="short bf16 "
                                                        "attention sums"):
                                nc.vector.tensor_reduce(
                                    out=accs[:, rtl:rth, 0:NH],
                                    in_=exT[:, 0:cc, 0:NH].rearrange(
                                        "p (n j) h -> p n h j", n=nt),
                                    axis=AX, op=OP.add)
                                nc.vector.tensor_reduce(
                                    out=accs[:, rtl:rth, NH:AW],
                                    in_=Wf[:, 0:cc, 0:NV].rearrange(
                                        "p (n j) f -> p n f j", n=nt),
                                    axis=AX, op=OP.add)

                    dsta = accT[k * PT : (k + 1) * PT, 0:AW].rearrange(
                        "(b p) v -> p b v", p=128)
                    nc.sync.dma_start(out=dsta[:, 0:49, :],
                                      in_=accs[:, 0:49, :])
                    nc.scalar.dma_start(out=dsta[:, 49:NT, :],
                                        in_=accs[:, 49:NT, :])

            # ---- combine pass (layer 1: -> t2shard+AllGather; 2: -> outp)
            def combine(layer):
                accT = acc1 if layer == 1 else acc2
                NH = 8 if layer == 1 else 1
                NV = HID if layer == 1 else OUT
                AW = NH + NV
                selfT = t1all if layer == 1 else t2all
                alo = 72 if layer == 1 else 3

                tot = apool.tile([128, NT, AW], bf16, tag=f"tot{layer}")
                for k in range(NCH):
                    cw0 = cmb_call_off[k][0][0]
                    cwn = sum(nn for (_, nn) in cmb_call_off[k]) // 16
                    cit = wpool.tile([128, IXW], i16, tag="ix")
                    nc.sync.dma_start(out=cit[:, 0:cwn],
                                      in_=gidx[:, cw0 : cw0 + cwn])
                    for ci, (wo, nn) in enumerate(cmb_call_off[k]):
                        gf = gpool.tile([128, NIMAX // 128, EW], bf16,
                                        tag="V", bufs=2)
                        lo = wo - cw0
                        nc.gpsimd.dma_gather(
                            gf[:, 0 : nn // 128, :],
                            accT[k * PT : (k + 1) * PT, :],
                            cit[:, lo : lo + nn // 16], nn, nn, EW,
                            queue_num=nextq())
                        b0 = ci * (NIMAX // 128)
                        nb = nn // 128
                        if k == 0:
                            nc.vector.tensor_copy(
                                out=tot[:, b0 : b0 + nb, :],
                                in_=gf[:, 0:nb, 0:AW])
                        else:
                            nc.vector.tensor_tensor(
                                out=tot[:, b0 : b0 + nb, :],
                                in0=tot[:, b0 : b0 + nb, :],
                                in1=gf[:, 0:nb, 0:AW], op=OP.add)

                o2 = None
                if layer == 2:
                    o2 = mpool.tile([128, NT, OUT], f32, tag="o2", bufs=1,
                                    name="o2")
                for h0 in range(0, NT, FG):
                    nb = min(FG, NT - h0)
                    sT = selfT[:, h0 : h0 + nb, :]
                    tT = tot[:, h0 : h0 + nb, :]
                    # self loop: e = al_src + al_dst (own row) -> exp
                    eS = mpool.tile([128, FG, 8], f32, tag="eS", bufs=1)
                    nc.vector.tensor_tensor(out=eS[:, 0:nb, 0:NH],
                                            in0=sT[:, :, 0:NH],
                                            in1=sT[:, :, alo : alo + NH],
                                            op=OP.add)
                    lkS = mpool.tile([128, FG, 8], f32, tag="lkS", bufs=1)
                    if use_prelu:
                        nc.scalar.activation(out=lkS[:, 0:nb, 0:NH],
                                             in_=eS[:, 0:nb, 0:NH],
                                             func=ACT.Prelu, alpha=NEG)
                    else:
                        nc.vector.tensor_scalar(out=lkS[:, 0:nb, 0:NH],
                                                in0=eS[:, 0:nb, 0:NH],
                                                scalar1=NEG, scalar2=None,
                                                op0=OP.mult)
                        nc.vector.tensor_tensor(out=lkS[:, 0:nb, 0:NH],
                                                in0=lkS[:, 0:nb, 0:NH],
                                                in1=eS[:, 0:nb, 0:NH],
                                                op=OP.max)
                    exS = mpool.tile([128, FG, 8], bf16, tag="exS", bufs=1)
                    nc.scalar.activation(out=exS[:, 0:nb, 0:NH],
                                         in_=lkS[:, 0:nb, 0:NH], func=ACT.Exp)
                    den = mpool.tile([128, FG, 8], f32, tag="den", bufs=1)
                    nc.vector.tensor_tensor(out=den[:, 0:nb, 0:NH],
                                            in0=tT[:, :, 0:NH],
                                            in1=exS[:, 0:nb, 0:NH], op=OP.add)
                    wS = mpool.tile([128, FG, HID], bf16, tag="wS", bufs=1)
                    if layer == 1:
                        nc.vector.tensor_tensor(
                            out=wS[:, 0:nb, :].rearrange(
                                "p b (h w) -> p b h w", h=NH),
                            in0=sT[:, :, NH : NH + NV].rearrange(
                                "p b (h w) -> p b h w", h=NH),
                            in1=exS[:, 0:nb, 0:NH].unsqueeze(3).to_broadcast(
                                [128, nb, NH, C1]),
                            op=OP.mult)
                    else:
                        nc.vector.tensor_tensor(
                            out=wS[:, 0:nb, 0:NV],
                            in0=sT[:, :, NH : NH + NV],
                            in1=exS[:, 0:nb, 0:1].to_broadcast([128, nb, NV]),
                            op=OP.mult)
                    num = mpool.tile([128, FG, HID], f32, tag="num", bufs=1)
                    nc.vector.tensor_tensor(out=num[:, 0:nb, 0:NV],
                                            in0=tT[:, :, NH:AW],
                                            in1=wS[:, 0:nb, 0:NV], op=OP.add)
                    rden = mpool.tile([128, FG, 8], f32, tag="rden", bufs=1)
                    nc.vector.reciprocal(out=rden[:, 0:nb, 0:NH],
                                         in_=den[:, 0:nb, 0:NH])
                    if layer == 2:
                        nc.vector.tensor_tensor(
                            out=o2[:, h0 : h0 + nb, :],
                            in0=num[:, 0:nb, 0:NV],
                            in1=rden[:, 0:nb, 0:1].to_broadcast(
                                [128, nb, OUT]),
                            op=OP.mult)
                        continue
                    # z = num/den, elu(z) = max(z,0) + exp(min(z,0)) - 1
                    z = mpool.tile([128, FG, HID], f32, tag="z", bufs=1)
                    nc.vector.tensor_tensor(
                        out=z[:, 0:nb, :].rearrange("p b (h w) -> p b h w",
                                                    h=NH),
                        in0=num[:, 0:nb, :].rearrange("p b (h w) -> p b h w",
                                                      h=NH),
                        in1=rden[:, 0:nb, 0:NH].unsqueeze(3).to_broadcast(
                            [128, nb, NH, C1]),
                        op=OP.mult)
                    zm = mpool.tile([128, FG, HID], f32, tag="zm", bufs=1)
                    nc.vector.tensor_scalar(out=zm[:, 0:nb, :],
                                            in0=z[:, 0:nb, :], scalar1=0.0,
                                            scalar2=None, op0=OP.min)
                    ze = mpool.tile([128, FG, HID], f32, tag="ze", bufs=1)
                    nc.scalar.activation(out=ze[:, 0:nb, :],
                                         in_=zm[:, 0:nb, :], func=ACT.Exp)
                    zb = mpool.tile([128, FG, HID], bf16, tag="zb", bufs=1)
                    nc.vector.tensor_scalar(out=zb[:, 0:nb, :],
                                            in0=z[:, 0:nb, :], scalar1=0.0,
                                            scalar2=None, op0=OP.max)
                    nc.vector.tensor_tensor(out=zb[:, 0:nb, :],
                                            in0=zb[:, 0:nb, :],
                                            in1=ze[:, 0:nb, :], op=OP.add)
                    nc.vector.tensor_scalar(out=zb[:, 0:nb, :],
                                            in0=zb[:, 0:nb, :], scalar1=-1.0,
                                            scalar2=None, op0=OP.add)
                    for b in range(nb):
                        zT = ps2pool.tile([HID, 128], bf16, tag="zT")
                        nc.tensor.transpose(out=zT[:], in_=zb[:, b, :],
                                            identity=ident[:])
                        zTs = mpool.tile([HID, 128], bf16, tag="zTs")
                        nc.vector.tensor_copy(out=zTs[:], in_=zT[:])
                        g2 = ps2pool.tile([128, 4], f32, tag="g2")
                        nc.tensor.matmul(out=g2[:], lhsT=zTs[:], rhs=w2es[:],
                                         start=True, stop=True)
                        nc.vector.tensor_tensor(out=t2all[:, h0 + b, :],
                                                in0=g2[:], in1=b2es[:],
                                                op=OP.add)
                if layer == 2:
                    dsto = outp[:].rearrange("p (b v) -> p b v", v=OUT)
                    nc.sync.dma_start(out=dsto, in_=o2[:])
                    return
                dst2 = t2shard[:, 0:4].rearrange("(b p) v -> p b v", p=128)
                nc.sync.dma_start(out=dst2[:, 0:49, :], in_=t2all[:, 0:49, :])
                nc.scalar.dma_start(out=dst2[:, 49:NT, :],
                                    in_=t2all[:, 49:NT, :])
                nc.gpsimd.collective_compute(
                    "AllGather", OP.bypass,
                    replica_groups=[list(range(NCORES))],
                    ins=[t2shard[:]], outs=[table2[:]],
                )

            t2all = cpool.tile([128, NT, 4], bf16)
            if phases in ("e1", "c1", "all"):
                edge_pass(1)
            if phases in ("c1", "all"):
                combine(1)
            if phases == "all":
                edge_pass(2)
                combine(2)
            if phases != "all":
                od = mpool.tile([128, NT, OUT], f32, tag="o2", bufs=1,
                                name="od")
                srcT = t2all if phases == "c1" else t1all
                nc.vector.tensor_copy(out=od[:], in_=srcT[:, :, 0:OUT])
                dsto = outp[:].rearrange("p (b v) -> p b v", v=OUT)
                nc.sync.dma_start(out=dsto, in_=od[:])

    nc.compile()
    return nc


def kernel(**inputs):
    from concourse.bass_utils import run_bass_kernel_spmd

    x = np.asarray(inputs["x"], dtype=np.float32)
    ei = np.asarray(inputs["edge_index"]).astype(np.int64)
    w1 = np.asarray(inputs["W1"], dtype=np.float32)
    a1s = np.asarray(inputs["a1_src"], dtype=np.float32)
    a1d = np.asarray(inputs["a1_dst"], dtype=np.float32)
    b1 = np.asarray(inputs["b1"], dtype=np.float32)
    w2 = np.asarray(inputs["W2"], dtype=np.float32)
    a2s = np.asarray(inputs["a2_src"], dtype=np.float32)
    a2d = np.asarray(inputs["a2_dst"], dtype=np.float32)
    b2 = np.asarray(inputs["b2"], dtype=np.float32)

    src, dst = ei[0], ei[1]
    per_core, sched, perm, gpos = _plan(src, dst)

    # weights: w1e = [W1@A1s | W1 | W1@A1d]  (al_src | h | al_dst columns)
    A1s = np.zeros((HID, H1), dtype=np.float32)
    A1d = np.zeros((HID, H1), dtype=np.float32)
    for h in range(H1):
        A1s[h * C1 : (h + 1) * C1, h] = a1s[h]
        A1d[h * C1 : (h + 1) * C1, h] = a1d[h]
    w1e = np.concatenate([w1 @ A1s, w1, w1 @ A1d], axis=1)      # [128, 80]
    w2e = np.concatenate([w2 @ a2s.T, w2, w2 @ a2d.T], axis=1)  # [64, 4]
    b1e = np.zeros((128, 80), dtype=np.float32)
    b1e[:, 8 : 8 + HID] = b1[None, :]
    b2e = np.zeros((128, 4), dtype=np.float32)
    b2e[:, 1 : 1 + OUT] = b2[None, :]

    import os
    phases = os.environ.get("GAT_PHASES", "all")
    key = (tuple(sched["K"].reshape(-1).tolist()), phases)
    if key not in _BUILD_CACHE:
        _BUILD_CACHE[key] = _build(sched, phases)
    nc = _BUILD_CACHE[key]

    common = dict(w1e=w1e.astype(BF16), b1e=b1e, w2e=w2e.astype(BF16),
                  b2e=b2e.astype(BF16))
    in_maps = []
    for c in range(NCORES):
        xp_c = np.zeros((PT, IN), dtype=np.float32)
        real = perm[c] >= 0
        xp_c[real] = x[c * ND + perm[c][real]]
        m = dict(common)
        m["xts"] = np.ascontiguousarray(xp_c.T).astype(BF16)
        m["gidx"] = per_core[c]["gidx"]
        m["padm"] = per_core[c]["padm"]
        in_maps.append(m)

    global _LAST_IN_MAPS
    _LAST_IN_MAPS = in_maps
    res = run_bass_kernel_spmd(nc, in_maps, list(range(NCORES)))

    out = np.zeros((N, OUT), dtype=np.float32)
    for c in range(NCORES):
        op = res.results[c]["outp"]       # [128, NT*2] partition-major
        op = op.reshape(128, NT, OUT).transpose(1, 0, 2).reshape(PT, OUT)
        real = perm[c] >= 0
        out[c * ND + perm[c][real]] = op[real]
    return out
